# revision 1
# baseline (speedup 1.0000x reference)
"""ACDA (adaptive conv) Trainium2 kernel — 8-core data-parallel over batch.

Per core: one sample (C=64, H=128, W=128). The image is split into two
64-row halves stacked on the 128 SBUF partitions (partition p<64 -> half A
channel p, p>=64 -> half B channel p-64), so every engine op runs with all
128 lanes active; the two halves' matmuls run concurrently in opposite
quadrants of the PE array (tile_position (0,0) / (64,64)).

Host-side prep (inside kernel(), not on device): x is zero-padded, bf16-cast
and laid out per-core as two copies (xpadA with a left pad column, xpadB
column-shifted by one) so that all nine 3x3-tap shifts are 4-byte-aligned
SBUF views; weights are pre-transposed to lhsT layout (c_in, k, c_out).

Pipeline per 8-row tile (per half), fully overlapped by the Tile scheduler:
  DMA: padded x arrives in 4 row-bands so compute starts after ~1/4 of input
  PE:  g_k = W_k^T @ x  (bf16 in, fp32 PSUM), per kernel-position k (9x)
  ACT: f_k = relu(g_k + b_k)  PSUM -> SBUF bf16  (bias is per-partition)
  DVE: prod_k = f_k * patch_k (bf16 2x_1P mode)
  DVE: out = sum_k prod_k (pairwise bf16 tree)
  DMA: out tile -> DRAM in bf16 (widened to fp32 on the host)

Measured on TRN2 (loop-amortized single-pass, 3-point wall regression):
~112 us; relative error vs the fp32 reference: 4.6e-3.
"""

import numpy as np
import ml_dtypes
from contextlib import ExitStack

import concourse.bass as bass
import concourse.tile as tile
from concourse import bacc, mybir
from concourse.bass_utils import run_bass_kernel_spmd

B, C, H, W, K = 8, 64, 128, 128, 3
NCORES = 8
RT = 8            # output rows per tile (per half)
NT = 64 // RT     # tiles per sample
BF16 = mybir.dt.bfloat16
F32 = mybir.dt.float32
RELU = mybir.ActivationFunctionType.Relu
MULT = mybir.AluOpType.mult
ADD = mybir.AluOpType.add
MAX = mybir.AluOpType.max

_CACHE = {}


def _window3(ap, lr0, dj, rt):
    """Overlapping 3-row window AP: [128, 3(di), rt, 128] over a padded-x
    band tile [128, nrows, rowlen], starting at buffer row lr0, col dj."""
    a = ap.copy()
    v = a.ap
    row_stride = v[1][0]
    v[1] = [row_stride, 3]
    v[2] = [row_stride, rt]
    v.append([1, 128])
    a.offset = a.offset + lr0 * row_stride + dj
    return a


def _kernel_body(ctx: ExitStack, tc, out_d, xA_d, xB_d, wT_d, bias_d, nreps=1,
                 opts=None):
    nc = tc.nc
    o = dict(rt=RT, evict="act", stage="full", psum_bufs=4, fbufs=2, pbufs=2,
             bands=True)
    o.update(opts or {})
    rt = o["rt"]
    nt = 64 // rt

    inp = ctx.enter_context(tc.tile_pool(name="inp", bufs=1))
    wT = inp.tile([128, 9, 64], BF16)            # lhsT per k, duplicated on halves
    bias = inp.tile([128, 9], F32)
    nc.sync.dma_start(wT[:], wT_d[:])
    nc.sync.dma_start(bias[:], bias_d[:])
    if o.get("bands"):
        # x staged in 4 row-bands (2 tiles + halo each) so the first matmuls
        # can start after ~1/4 of the input DMA.
        xAb = [inp.tile([128, 18, 130], BF16, name=f"xAb{b}", tag=f"xA{b}")
               for b in range(4)]
        xBb = [inp.tile([128, 18, 128], BF16, name=f"xBb{b}", tag=f"xB{b}")
               for b in range(4)]
        for b in range(4):
            nc.sync.dma_start(xAb[b][:], xA_d[:, 16 * b: 16 * b + 18, :])
            nc.sync.dma_start(xBb[b][:], xB_d[:, 16 * b: 16 * b + 18, :])
    else:
        xAw = inp.tile([128, 66, 130], BF16)
        xBw = inp.tile([128, 66, 128], BF16)
        nc.sync.dma_start(xAw[:], xA_d[:])
        nc.sync.dma_start(xBw[:], xB_d[:])
        xAb = [xAw[:, 16 * b: 16 * b + 18, :] for b in range(4)]
        xBb = [xBw[:, 16 * b: 16 * b + 18, :] for b in range(4)]

    psum = ctx.enter_context(tc.tile_pool(name="psum", bufs=o["psum_bufs"], space="PSUM"))
    o["_ftmp_pool"] = ctx.enter_context(tc.tile_pool(name="ftmp", bufs=4))
    fpool = ctx.enter_context(tc.tile_pool(name="f", bufs=o["fbufs"]))
    ppool = ctx.enter_context(tc.tile_pool(name="prod", bufs=o["pbufs"]))
    apool = ctx.enter_context(tc.tile_pool(name="a", bufs=2))
    bpool = ctx.enter_context(tc.tile_pool(name="b", bufs=2))
    cpool = ctx.enter_context(tc.tile_pool(name="c", bufs=2))
    opool = ctx.enter_context(tc.tile_pool(name="o", bufs=2))

    out4 = out_d.rearrange("c (h r) w -> h c r w", h=2)

    fdt = F32 if o["stage"] == "evict_f32" else BF16
    assert rt == 8, "row-band staging assumes rt=8"
    for t in range(nt * nreps):
        r0 = (t % nt) * rt
        band = (t % nt) // 2
        lr0 = r0 - 16 * band
        xA, xB = xAb[band], xBb[band]
        f = fpool.tile([128, 9, rt, 128], fdt)
        for k in range(9):
            ps = psum.tile([128, rt, 128], F32)
            if o["stage"] != "nomm":
                for h in (0, 1):
                    p0 = 64 * h
                    # center pixels: buffer row r+1, buffer cols 1..128
                    for n in range(rt // 4):  # <=512 fp32 cols per matmul (1 bank)
                        rhs = xA[p0:p0 + 64, lr0 + 1 + 4 * n: lr0 + 5 + 4 * n, 1:129]
                        nc.tensor.matmul(
                            ps[p0:p0 + 64, 4 * n: 4 * n + 4, :],
                            wT[p0:p0 + 64, k, :],
                            rhs,
                            start=True, stop=True,
                        )
            ev = o["evict"]
            if ev == "cpbf16":
                nc.scalar.activation(f[:, k], ps[:],
                                     mybir.ActivationFunctionType.Copy, bias=0.0)
            elif ev == "ts1":
                nc.vector.tensor_scalar_max(f[:, k], ps[:], 0.0)
            elif ev == "twopass":
                ftmp = o["_ftmp_pool"].tile([128, rt, 128], F32, tag="ftmp")
                nc.scalar.activation(ftmp[:], ps[:], RELU, bias=bias[:, k:k + 1])
                nc.scalar.activation(f[:, k], ftmp[:],
                                     mybir.ActivationFunctionType.Copy, bias=0.0)
            elif ev == "gpcvt" and k >= o.get("gp_k0", 3):
                ftmp = o["_ftmp_pool"].tile([128, rt, 128], F32, tag="ftmp",
                                            name=f"ftmp_{t}_{k}")
                nc.scalar.activation(ftmp[:], ps[:], RELU, bias=bias[:, k:k + 1])
                nc.gpsimd.tensor_copy(f[:, k], ftmp[:])
            elif ev == "mix1" and k == 4:
                nc.vector.tensor_scalar(f[:, k], ps[:], bias[:, k:k + 1], 0.0,
                                        op0=ADD, op1=MAX)
            elif ev in ("act", "mix1", "gpcvt") or (ev == "mix" and k % 3 != 2):
                nc.scalar.activation(f[:, k], ps[:], RELU, bias=bias[:, k:k + 1])
            else:
                nc.vector.tensor_scalar(f[:, k], ps[:], bias[:, k:k + 1], 0.0,
                                        op0=ADD, op1=MAX)

        if o["stage"] in ("evict_only", "evict_f32", "nomm"):
            ob = opool.tile([128, rt, 128], BF16)
            nc.vector.tensor_tensor(ob[:], f[:, 0], f[:, 8], op=ADD)
            nc.sync.dma_start(out4[:, :, r0:r0 + rt, :], ob[:])
            continue

        prod = ppool.tile([128, 9, rt, 128], BF16)
        if o.get("m3"):
            # one DVE op per dj: di rides the access pattern (stride = one
            # buffer row), so 3 taps are multiplied per instruction
            fr = f[:].rearrange("p (di dj) r c -> p di dj r c", dj=3)
            pr = prod[:].rearrange("p (di dj) r c -> p di dj r c", dj=3)
            for dj in range(3):
                if dj == 1:
                    patch = _window3(xB[:], lr0, 0, rt)
                else:
                    patch = _window3(xA[:], lr0, dj, rt)
                nc.vector.tensor_tensor(pr[:, :, dj], fr[:, :, dj], patch,
                                        op=MULT)
        else:
            for k in range(9):
                di, dj = divmod(k, 3)
                if dj == 1:
                    patch = xB[:, lr0 + di: lr0 + di + rt, 0:128]
                else:
                    patch = xA[:, lr0 + di: lr0 + di + rt, dj:dj + 128]
                nc.vector.tensor_tensor(prod[:, k], f[:, k], patch, op=MULT)

        if o["stage"] == "noadd":
            ob = opool.tile([128, rt, 128], BF16)
            nc.vector.tensor_tensor(ob[:], prod[:, 0], prod[:, 8], op=ADD)
            nc.sync.dma_start(out4[:, :, r0:r0 + rt, :], ob[:])
            continue

        a = apool.tile([128, 4, rt, 128], BF16)
        nc.vector.tensor_tensor(a[:], prod[:, 0:4], prod[:, 4:8], op=ADD)
        b2 = bpool.tile([128, 2, rt, 128], BF16)
        nc.vector.tensor_tensor(b2[:], a[:, 0:2], a[:, 2:4], op=ADD)
        c = cpool.tile([128, rt, 128], BF16)
        nc.vector.tensor_tensor(c[:], b2[:, 0], b2[:, 1], op=ADD)
        ob = opool.tile([128, rt, 128], F32 if o.get("o32") else BF16)
        nc.vector.tensor_tensor(ob[:], c[:], prod[:, 8], op=ADD)

        nc.sync.dma_start(out4[:, :, r0:r0 + rt, :], ob[:])


def _build():
    if "nc" in _CACHE:
        return _CACHE["nc"]
    nc = bacc.Bacc("TRN2", target_bir_lowering=False, debug=False,
                   num_devices=NCORES)
    xA_d = nc.dram_tensor("xpadA", (128, 66, 130), BF16, kind="ExternalInput").ap()
    xB_d = nc.dram_tensor("xpadB", (128, 66, 128), BF16, kind="ExternalInput").ap()
    wT_d = nc.dram_tensor("wT", (128, 9, 64), BF16, kind="ExternalInput").ap()
    bias_d = nc.dram_tensor("bias", (128, 9), F32, kind="ExternalInput").ap()
    out_d = nc.dram_tensor("out", (C, H, W), BF16, kind="ExternalOutput").ap()
    with tile.TileContext(nc) as tc, ExitStack() as ctx:
        _kernel_body(ctx, tc, out_d, xA_d, xB_d, wT_d, bias_d)
    nc.compile()
    _CACHE["nc"] = nc
    return nc


def _prep_core_inputs(x_i: np.ndarray, wT_np, bias_np):
    """x_i: (C, H, W) float32 -> per-core input dict."""
    bf = ml_dtypes.bfloat16
    xA = np.zeros((128, 66, 130), dtype=bf)
    xB = np.zeros((128, 66, 128), dtype=bf)
    xb = x_i.astype(bf)
    # half A: buffer rows 0..65 = x rows -1..64 (row -1 zero-padded)
    xA[0:64, 1:66, 1:129] = xb[:, 0:65, :]
    xB[0:64, 1:66, :] = xb[:, 0:65, :]
    # half B: buffer rows 0..65 = x rows 63..128 (row 128 zero-padded)
    xA[64:128, 0:65, 1:129] = xb[:, 63:128, :]
    xB[64:128, 0:65, :] = xb[:, 63:128, :]
    return {"xpadA": xA, "xpadB": xB, "wT": wT_np, "bias": bias_np}


def kernel(x: np.ndarray, W_gen: np.ndarray, b_gen: np.ndarray) -> np.ndarray:
    x = np.asarray(x, dtype=np.float32)
    W_gen = np.asarray(W_gen, dtype=np.float32)
    b_gen = np.asarray(b_gen, dtype=np.float32)

    nc = _build()

    bf = ml_dtypes.bfloat16
    # lhsT: (c_in, k, c_out); o index in reference = c_out*9 + k
    wT_half = W_gen.reshape(C, K * K, C).transpose(2, 1, 0).astype(bf)  # (cin,k,cout)
    wT_np = np.ascontiguousarray(np.concatenate([wT_half, wT_half], axis=0))
    b2 = b_gen.reshape(C, K * K).astype(np.float32)                     # (c_out, k)
    bias_np = np.ascontiguousarray(np.concatenate([b2, b2], axis=0))    # (128, 9)

    in_maps = [_prep_core_inputs(x[i], wT_np, bias_np) for i in range(NCORES)]
    res = run_bass_kernel_spmd(nc, in_maps, core_ids=list(range(NCORES)))
    out = np.stack([res.results[i]["out"] for i in range(NCORES)], axis=0)
    return out.astype(np.float32)


if __name__ == "__main__":
    xs = np.random.randn(B, C, H, W).astype(np.float32)
    Wg = np.random.randn(C * K * K, C).astype(np.float32) / np.sqrt(C)
    bg = (np.random.randn(C * K * K) * 0.01).astype(np.float32)
    o = kernel(xs, Wg, bg)
    print("out", o.shape, o.dtype, float(np.abs(o).mean()))

